# revision 1
# baseline (speedup 1.0000x reference)
"""Trainium2 Bass kernel for the DiscretizedDPLRSSMBlock problem.

Computes, for h, x of shape [4096, 4096] (batch, hidden):

    out = h + (h * a_diag + (h @ q_vec) @ p_vec.T) + x @ b_mat        (DELTA = 1.0)
        = h * (1 + a_diag) + (h @ q_vec) @ p_vec.T + x @ b_mat

Sharding: data-parallel over the batch axis across 8 NeuronCores (512 rows
per core); a_diag/p_vec/q_vec/b_mat replicated.

Per-core kernel works in a transposed layout (hidden on partitions):
    outT[n, m] = sum_k B[k, n] * xT[k, m]        (x @ B, B tiles are the
                                                  stationary matmul operand
                                                  in natural DRAM layout)
               + sum_r p[n, r] * hqT[r, m]       (rank-4 term, hqT = q^T hT)
               + (1 + a[n]) * hT[n, m]           (per-partition scalar on DVE)

All matmul operands are bf16 (fp32 PSUM accumulation); output is fp32.
"""

import numpy as np
import ml_dtypes

import concourse.mybir as mybir
import concourse.tile as tile
from concourse import bacc
from concourse.bass_utils import run_bass_kernel_spmd

HIDDEN = 4096
BATCH = 4096
RANK = 4
N_CORES = 8
MB = BATCH // N_CORES  # 512 batch rows per core
P = 128
KT = HIDDEN // P       # 32 contraction tiles
NT = HIDDEN // P       # 32 output row tiles (hidden)
NCHUNK = 4             # resident tensors split into 4 DMA chunks
CH = KT // NCHUNK      # 8 k-tiles per chunk
NGROUP = NT // 8       # 4 n-tiles per b-column streaming group (512 cols)

BF16 = mybir.dt.bfloat16
F32 = mybir.dt.float32


def build_bass():
    """Build the single-core Tile program (same program runs SPMD on all 8)."""
    nc = bacc.Bacc("TRN2", target_bir_lowering=False, debug=False)

    b = nc.dram_tensor("b", [HIDDEN, HIDDEN], BF16, kind="ExternalInput")
    xT = nc.dram_tensor("xT", [HIDDEN, MB], BF16, kind="ExternalInput")
    hT = nc.dram_tensor("hT", [HIDDEN, MB], BF16, kind="ExternalInput")
    q = nc.dram_tensor("q", [HIDDEN, RANK], BF16, kind="ExternalInput")
    pT = nc.dram_tensor("pT", [RANK, HIDDEN], BF16, kind="ExternalInput")
    a_r = nc.dram_tensor("a_r", [P, NT], F32, kind="ExternalInput")
    outT = nc.dram_tensor("outT", [HIDDEN, MB], F32, kind="ExternalOutput")

    b_r = b.rearrange("(t p) n -> p t n", p=P)     # [128, 32, 4096]
    xT_r = xT.rearrange("(t p) m -> p t m", p=P)   # [128, 32, 512]
    hT_r = hT.rearrange("(t p) m -> p t m", p=P)
    q_r = q.rearrange("(t p) r -> p t r", p=P)     # [128, 32, 4]

    # Episode chunking over the 32 k-tiles. 1MB-granularity transfers keep
    # the single HWDGE ring at full rate; finer chunks measured slower.
    CHUNKS = [(0, 8), (8, 8), (16, 8), (24, 8)]  # (t0, len)
    NEP = len(CHUNKS)

    with (
        tile.TileContext(nc) as tc,
        tc.tile_pool(name="const", bufs=1) as cpool,
        tc.tile_pool(name="bcols", bufs=3) as bpool,
        tc.tile_pool(name="psum", bufs=6, space="PSUM") as pspool,
        tc.tile_pool(name="outs", bufs=4) as opool,
    ):
        n_groups = NT // NGROUP

        def dma_b_group(g):
            n0 = g * NGROUP * P
            bcs = []
            for c, (t0, ln) in enumerate(CHUNKS):
                bc = bpool.tile(
                    [P, ln, NGROUP * P], BF16, tag=f"b{c}", name=f"b{g}_{c}"
                )
                nc.sync.dma_start(bc[:], b_r[:, t0 : t0 + ln, n0 : n0 + NGROUP * P])
                bcs.append(bc)
            return bcs

        # ---- DMA issue order chosen to match PE consumption order ----
        # Interleave b-group0 / x chunks (PE's first ~28us), with hT woven
        # into the tail so hq can start right when group 0's mains finish.
        # All input DMAs ride the Sync HWDGE ring; output DMAs ride the
        # Scalar ring so they can never head-of-line-block input streaming.
        # Issue order: pure b/x for the first two episodes, then weave hT
        # chunks (for the hq prologue) into the tail — each lands just
        # before its interleaved hq chunk-matmuls need it.
        xc, hc = [], []
        bcs0 = []

        def dma_x(c):
            t0, ln = CHUNKS[c]
            xt = cpool.tile([P, ln, MB], BF16, tag=f"x{c}")
            nc.sync.dma_start(xt[:], xT_r[:, t0 : t0 + ln, :])
            xc.append(xt)

        def dma_b0(c):
            t0, ln = CHUNKS[c]
            bc = bpool.tile([P, ln, NGROUP * P], BF16, tag=f"b{c}", name=f"b0_{c}")
            nc.sync.dma_start(bc[:], b_r[:, t0 : t0 + ln, 0 : NGROUP * P])
            bcs0.append(bc)

        def dma_h(cc):
            ht = cpool.tile([P, CH, MB], BF16, tag=f"h{cc}", name=f"h{cc}")
            nc.sync.dma_start(ht[:], hT_r[:, cc * CH : (cc + 1) * CH, :])
            hc.append(ht)

        dma_b0(0); dma_x(0); dma_b0(1); dma_x(1)
        dma_h(0)
        q_sb = cpool.tile([P, KT, RANK], BF16, tag="q")
        nc.sync.dma_start(q_sb[:], q_r[:])
        dma_b0(2); dma_x(2)
        dma_h(1)
        dma_b0(3); dma_x(3)
        dma_h(2); dma_h(3)
        # rank-4 operands zero-padded to K=128: a K=4 LDWEIGHTS targets only
        # row-group 0 and conflicts with in-flight full-array matmuls (no
        # overlap); full-height loads pipeline normally.
        pT_sb = cpool.tile([P, HIDDEN], BF16, tag="pT")
        nc.any.memset(pT_sb[:], 0.0)
        nc.sync.dma_start(pT_sb[0:RANK, :], pT[:, :])
        araw = cpool.tile([P, NT], F32, tag="araw")
        nc.sync.dma_start(araw[:], a_r[:, :])
        a1 = cpool.tile([P, NT], F32, tag="a1")
        nc.vector.tensor_scalar_add(a1[:], araw[:], 1.0)

        def sub_epilogue(tn, ps):
            ot = opool.tile([P, MB], F32, tag="ot", name=f"ot{tn}")
            nc.vector.scalar_tensor_tensor(
                ot[:],
                hc[tn // CH][:, tn % CH],
                a1[:, tn : tn + 1],
                ps[:],
                mybir.AluOpType.mult,
                mybir.AluOpType.add,
            )
            nc.scalar.dma_start(outT[tn * P : (tn + 1) * P, :], ot[:])

        def rank4(tn, ps):
            nc.tensor.matmul(
                ps[:],
                pT_sb[:, tn * P : (tn + 1) * P],
                hq_sb[:],
                start=False,
                stop=True,
            )

        def main_episodes(g, bcs, pss, tail_inline):
            # k-outer: episode c consumes exactly (b chunk c, x chunk c),
            # matching DMA delivery order; 4 psum banks accumulate.
            for c, (t0, ln) in enumerate(CHUNKS):
                last = c == NEP - 1
                for sub in range(NGROUP):
                    for tt in range(ln):
                        nc.tensor.matmul(
                            pss[sub][:],
                            bcs[c][:, tt, sub * P : (sub + 1) * P],
                            xc[c][:, tt],
                            start=(c == 0 and tt == 0),
                            stop=False,
                        )
                    if last and tail_inline:
                        tn = g * NGROUP + sub
                        rank4(tn, pss[sub])
                        sub_epilogue(tn, pss[sub])

        # ---- group 0: mains with the hq prologue (hqT = q^T @ hT, [4,512])
        # chunk-interleaved between episodes as each hT chunk lands ----
        pss0 = [
            pspool.tile([P, MB], F32, tag="ps", name=f"ps0_{i}")
            for i in range(NGROUP)
        ]
        hq_ps = pspool.tile([RANK, MB], F32, tag="hq", bufs=1)

        def g0_episode(c):
            t0, ln = CHUNKS[c]
            for sub in range(NGROUP):
                for tt in range(ln):
                    nc.tensor.matmul(
                        pss0[sub][:],
                        bcs0[c][:, tt, sub * P : (sub + 1) * P],
                        xc[c][:, tt],
                        start=(c == 0 and tt == 0),
                        stop=False,
                    )

        def hq_chunk(cc):
            for tt in range(CH):
                nc.tensor.matmul(
                    hq_ps[:],
                    q_sb[:, cc * CH + tt],
                    hc[cc][:, tt],
                    start=(cc == 0 and tt == 0),
                    stop=(cc == 3 and tt == CH - 1),
                )

        g0_episode(0)
        g0_episode(1)
        hq_chunk(0)
        g0_episode(2)
        hq_chunk(1)
        g0_episode(3)
        hq_chunk(2)
        hq_chunk(3)

        hq_sb = cpool.tile([P, MB], BF16, tag="hq_sb")
        nc.any.memset(hq_sb[:], 0.0)
        nc.vector.tensor_copy(hq_sb[0:RANK, :], hq_ps[:])

        for sub in range(NGROUP):
            rank4(sub, pss0[sub])
        for sub in range(NGROUP):
            sub_epilogue(sub, pss0[sub])

        # ---- groups 1..7: inline rank4 + epilogue in the last episode ----
        for g in range(1, n_groups):
            bcs = dma_b_group(g)
            pss = [
                pspool.tile([P, MB], F32, tag="ps", name=f"ps{g}_{i}")
                for i in range(NGROUP)
            ]
            main_episodes(g, bcs, pss, tail_inline=True)

    nc.compile()
    return nc


_NC_CACHE = []


def _get_nc():
    if not _NC_CACHE:
        _NC_CACHE.append(build_bass())
    return _NC_CACHE[0]


LAST_RESULTS = []  # stash of the last BassKernelResults, for test harnesses


def make_in_maps(h, x, a_diag, p_vec, q_vec, b_mat):
    """Shard + lay out the full inputs into per-core in_maps."""
    h = np.asarray(h, dtype=np.float32)
    x = np.asarray(x, dtype=np.float32)
    a_diag = np.asarray(a_diag, dtype=np.float32)
    p_vec = np.asarray(p_vec, dtype=np.float32)
    q_vec = np.asarray(q_vec, dtype=np.float32)
    b_mat = np.asarray(b_mat, dtype=np.float32)

    bf = ml_dtypes.bfloat16
    b_bf = np.ascontiguousarray(b_mat.astype(bf))
    q_bf = np.ascontiguousarray(q_vec.astype(bf))
    pT_bf = np.ascontiguousarray(p_vec.T.astype(bf))
    # a_r[p, t] = a_diag[t*128 + p]
    a_r = np.ascontiguousarray(a_diag.reshape(NT, P).T)

    in_maps = []
    for c in range(N_CORES):
        sl = slice(c * MB, (c + 1) * MB)
        in_maps.append(
            {
                "b": b_bf,
                "xT": np.ascontiguousarray(x[sl].T.astype(bf)),
                "hT": np.ascontiguousarray(h[sl].T.astype(bf)),
                "q": q_bf,
                "pT": pT_bf,
                "a_r": a_r,
            }
        )
    return in_maps


def _axon_device_reset():
    """Best-effort heal of a wedged axon-tunneled device (NRT_EXEC_UNIT_
    UNRECOVERABLE). No-op when the axon .so isn't present."""
    try:
        import ctypes

        lib = ctypes.CDLL("/opt/axon/libaxon_pjrt.so")
        lib.axon_reset.restype = ctypes.c_int64
        lib.axon_reset()
    except Exception:
        pass


def kernel(h, x, a_diag, p_vec, q_vec, b_mat, trace=False):
    nc = _get_nc()
    in_maps = make_in_maps(h, x, a_diag, p_vec, q_vec, b_mat)
    try:
        res = run_bass_kernel_spmd(
            nc, in_maps, core_ids=list(range(N_CORES)), trace=trace
        )
    except Exception as e:
        if "UNRECOVERABLE" not in str(e) and "UNAVAILABLE" not in str(e):
            raise
        _axon_device_reset()
        res = run_bass_kernel_spmd(
            nc, in_maps, core_ids=list(range(N_CORES)), trace=trace
        )
    LAST_RESULTS.clear()
    LAST_RESULTS.append(res)

    out = np.empty((BATCH, HIDDEN), dtype=np.float32)
    for c in range(N_CORES):
        out[c * MB : (c + 1) * MB, :] = res.results[c]["outT"].T
    return out



# revision 2
# speedup vs baseline: 1.2202x; 1.2202x over previous
"""Trainium2 Bass kernel for the DiscretizedDPLRSSMBlock problem (fp8 DoubleRow).

Computes, for h, x of shape [4096, 4096] (batch, hidden):

    out = h + (h * a_diag + (h @ q_vec) @ p_vec.T) + x @ b_mat        (DELTA = 1.0)

Sharding: data-parallel over the batch axis across 8 NeuronCores (512 rows
per core); a_diag/p_vec/q_vec/b_mat replicated.

The dominant x @ b_mat matmul runs in fp8 e4m3 with DoubleRow perf mode
(2 fp8 weights per PE cell, contraction 256 per pass). Host-side scaling:
x*SX and b*SB are quantized to e4m3, so PSUM holds SX*SB*(x@b). The rank-4
path's q is pre-scaled by SX*SB and a_r holds SX*SB*(1+a), so the entire
output is uniformly scaled by SX*SB = 2^15; the host divides it back out.

Per-core layout (hidden on partitions):
    outT[n, m] = sum_k B[k, n] * xT[k, m]        (fp8 DoubleRow, k in pairs
                                                  of 128-tiles)
               + sum_r p[n, r] * hqT[r, m]       (rank-4 term, bf16)
               + (1 + a[n]) * hT[n, m]           (per-partition scalar on DVE)
"""

import numpy as np
import ml_dtypes

import concourse.mybir as mybir
import concourse.tile as tile
from concourse import bacc
from concourse.bass_utils import run_bass_kernel_spmd

HIDDEN = 4096
BATCH = 4096
RANK = 4
N_CORES = 8
MB = BATCH // N_CORES  # 512 batch rows per core
P = 128
KT = HIDDEN // P       # 32 contraction tiles
NT = HIDDEN // P       # 32 output row tiles (hidden)
NCHUNK = 4             # resident tensors split into 4 DMA chunks
CH = KT // NCHUNK      # 8 k-tiles per chunk
NGROUP = NT // 8       # 4 n-tiles per b-column streaming group (512 cols)

SX = 16.0              # x fp8 scale (also used for h in the hq path)
SB = 2048.0            # b fp8 scale (also used for q in the hq path)
SOUT = SX * SB         # 2^15: uniform output scale, divided out on host
RPAD = 16              # rank padded to 16 so the DoubleRow pair step is %16

BF16 = mybir.dt.bfloat16
F32 = mybir.dt.float32
FP8 = mybir.dt.float8e4
DR = mybir.MatmulPerfMode.DoubleRow


def build_bass():
    """Build the single-core Tile program (same program runs SPMD on all 8)."""
    nc = bacc.Bacc("TRN2", target_bir_lowering=False, debug=False)

    # All heavy inputs are host-prepacked so every chunk DMA reads one fully
    # contiguous DRAM block (4 KB per-partition lines, no strided segments).
    NG = NT // NGROUP
    b = nc.dram_tensor("b", [NG, NCHUNK, P, CH, NGROUP * P], FP8,
                       kind="ExternalInput")
    xT = nc.dram_tensor("xT", [NCHUNK, P, CH, MB], FP8, kind="ExternalInput")
    hT = nc.dram_tensor("hT", [NCHUNK, P, CH, MB], BF16, kind="ExternalInput")
    hT8 = nc.dram_tensor("hT8", [NCHUNK, P, CH, MB], FP8, kind="ExternalInput")
    q8 = nc.dram_tensor("q8", [P, KT, RPAD], FP8, kind="ExternalInput")
    pT = nc.dram_tensor("pT", [RANK, HIDDEN], BF16, kind="ExternalInput")
    a_r = nc.dram_tensor("a_r", [P, NT], F32, kind="ExternalInput")
    outT = nc.dram_tensor("outT", [HIDDEN, MB], F32, kind="ExternalOutput")

    # Episode chunking over the 32 k-tiles. Chunks are even-sized so the
    # DoubleRow pair dim (adjacent k-tiles) never straddles a chunk.
    CHUNKS = [(0, 8), (8, 8), (16, 8), (24, 8)]  # (t0, len)
    NEP = len(CHUNKS)

    with (
        tile.TileContext(nc) as tc,
        tc.tile_pool(name="const", bufs=1) as cpool,
        tc.tile_pool(name="bcols", bufs=4) as bpool,
        tc.tile_pool(name="psum", bufs=7, space="PSUM") as pspool,
        tc.tile_pool(name="outs", bufs=4) as opool,
    ):
        n_groups = NT // NGROUP

        def dma_b_group(g):
            bcs = []
            for c in range(NCHUNK):
                bc = bpool.tile(
                    [P, CH, NGROUP * P], FP8, tag=f"b{c}", name=f"b{g}_{c}"
                )
                nc.sync.dma_start(bc[:], b[g, c])
                bcs.append(bc)
            return bcs

        # ---- DMA issue order chosen to match PE consumption order ----
        # HBM bandwidth is shared (~380 GB/s/core) across all HWDGE rings,
        # so the heavy b+x stream rides the Sync ring ALONE, in exactly the
        # order the PE consumes it (x and b chunks interleaved per episode).
        # The small hq-path inputs (q8, h8) ride the GpSimd ring; h (bf16,
        # needed only by the epilogues) rides the Scalar ring, with chunks
        # 1..3 issued behind earlier groups' output DMAs so they never steal
        # early bandwidth from b. Output DMAs ride the Scalar ring.
        xc, hc, hc8 = {}, [], []
        bcs0 = {}

        def dma_x(c):
            xt = cpool.tile([P, CH, MB], FP8, tag=f"x{c}")
            nc.sync.dma_start(xt[:], xT[c])
            xc[c] = xt

        def dma_b0(c):
            bc = bpool.tile([P, CH, NGROUP * P], FP8, tag=f"b{c}", name=f"b0_{c}")
            nc.sync.dma_start(bc[:], b[0, c])
            bcs0[c] = bc

        def dma_h(cc):
            ht = cpool.tile([P, CH, MB], BF16, tag=f"h{cc}", name=f"h{cc}")
            nc.scalar.dma_start(ht[:], hT[cc])
            hc.append(ht)

        def dma_h8(cc):
            ht = cpool.tile([P, CH, MB], FP8, tag=f"h8_{cc}", name=f"h8_{cc}")
            nc.sync.dma_start(ht[:], hT8[cc])
            hc8.append(ht)

        # The sync ring carries everything the PE consumes, in consumption
        # order, so arbitration with other rings never delays the next
        # operand. h (scalar ring) is gated behind the hq_sb copy below.
        dma_x(0); dma_b0(0)
        dma_b0(1); dma_x(1)
        dma_b0(2); dma_x(2)
        q_sb = cpool.tile([P, KT, RPAD], FP8, tag="q")
        nc.sync.dma_start(q_sb[:], q8[:])
        dma_h8(0)
        dma_b0(3); dma_x(3)
        dma_h8(1); dma_h8(2); dma_h8(3)
        # rank-4 operands zero-padded to K=128: a K=4 LDWEIGHTS targets only
        # row-group 0 and conflicts with in-flight full-array matmuls (no
        # overlap); full-height loads pipeline normally.
        pT_sb = cpool.tile([P, HIDDEN], BF16, tag="pT")
        nc.vector.memset(pT_sb[:], 0.0)
        nc.gpsimd.dma_start(pT_sb[0:RANK, :], pT[:, :])
        a1 = cpool.tile([P, NT], F32, tag="a1")
        nc.gpsimd.dma_start(a1[:], a_r[:, :])

        def sub_epilogue(tn, ps):
            ot = opool.tile([P, MB], F32, tag="ot", name=f"ot{tn}")
            nc.vector.scalar_tensor_tensor(
                ot[:],
                hc[tn // CH][:, tn % CH],
                a1[:, tn : tn + 1],
                ps[:],
                mybir.AluOpType.mult,
                mybir.AluOpType.add,
            )
            nc.scalar.dma_start(outT[tn * P : (tn + 1) * P, :], ot[:])

        def rank4(tn, ps, start, stop):
            nc.tensor.matmul(
                ps[:],
                pT_sb[:, tn * P : (tn + 1) * P],
                hq_sb[:],
                start=start,
                stop=stop,
            )

        def main_episodes(g, bcs, pss, seed):
            # k-outer: episode c consumes exactly (b chunk c, x chunk c),
            # matching DMA delivery order; 4 psum banks accumulate.
            # DoubleRow: each matmul contracts a pair of adjacent k-tiles.
            # seed=True: the rank-4 matmul opens each bank's accumulation
            # group (instead of closing it), so the tail after the last main
            # matmul is only the DVE epilogue + output DMA.
            if seed:
                for sub in range(NGROUP):
                    rank4(g * NGROUP + sub, pss[sub], start=True, stop=False)
            for c, (t0, ln) in enumerate(CHUNKS):
                last = c == NEP - 1
                for sub in range(NGROUP):
                    for uu in range(ln // 2):
                        nc.tensor.matmul(
                            pss[sub][:],
                            bcs[c][:, 2 * uu : 2 * uu + 2, sub * P : (sub + 1) * P],
                            xc[c][:, 2 * uu : 2 * uu + 2],
                            start=(not seed and c == 0 and uu == 0),
                            stop=(seed and last and uu == ln // 2 - 1),
                            perf_mode=DR,
                        )
                    if last:
                        tn = g * NGROUP + sub
                        if not seed:
                            rank4(tn, pss[sub], start=False, stop=True)
                        sub_epilogue(tn, pss[sub])

        # ---- group 0: mains with the hq prologue (hqT = q^T @ hT, [4,512])
        # chunk-interleaved between episodes as each hT chunk lands ----
        pss0 = [
            pspool.tile([P, MB], F32, tag="ps", name=f"ps0_{i}")
            for i in range(NGROUP)
        ]
        hq_ps = pspool.tile([RPAD, MB], F32, tag="hq", bufs=1)

        def g0_episode(c):
            t0, ln = CHUNKS[c]
            for sub in range(NGROUP):
                for uu in range(ln // 2):
                    nc.tensor.matmul(
                        pss0[sub][:],
                        bcs0[c][:, 2 * uu : 2 * uu + 2, sub * P : (sub + 1) * P],
                        xc[c][:, 2 * uu : 2 * uu + 2],
                        start=(c == 0 and uu == 0),
                        stop=False,
                        perf_mode=DR,
                    )

        def hq_chunk(cc):
            for uu in range(CH // 2):
                nc.tensor.matmul(
                    hq_ps[:],
                    q_sb[:, cc * CH + 2 * uu : cc * CH + 2 * uu + 2, :],
                    hc8[cc][:, 2 * uu : 2 * uu + 2],
                    start=(cc == 0 and uu == 0),
                    stop=(cc == 3 and uu == CH // 2 - 1),
                    perf_mode=DR,
                )

        g0_episode(0)
        g0_episode(1)
        hq_chunk(0)
        g0_episode(2)
        hq_chunk(1)
        g0_episode(3)
        hq_chunk(2)
        hq_chunk(3)

        # The hq_sb copy runs on the Scalar engine: it gates the scalar
        # ring's h DMAs (queued right after it) so they can't steal early
        # HBM bandwidth from the b/x stream on sync.
        hq_sb = cpool.tile([P, MB], BF16, tag="hq_sb")
        nc.vector.memset(hq_sb[:], 0.0)
        nc.scalar.copy(hq_sb[0:RPAD, :], hq_ps[:])
        dma_h(0)

        for sub in range(NGROUP):
            rank4(sub, pss0[sub], start=False, stop=True)
        for sub in range(NGROUP):
            sub_epilogue(sub, pss0[sub])
        dma_h(1)

        # ---- groups 1..7 ----
        # g1 keeps the tail rank4 (the hq copy may not be ready when its
        # banks open); g2+ seed their banks with rank4 up front.
        for g in range(1, n_groups):
            bcs = dma_b_group(g)
            pss = [
                pspool.tile([P, MB], F32, tag="ps", name=f"ps{g}_{i}")
                for i in range(NGROUP)
            ]
            main_episodes(g, bcs, pss, seed=(g >= 2))
            if g == 2:
                dma_h(2)
            elif g == 4:
                dma_h(3)

    nc.compile()
    return nc


_NC_CACHE = []


def _get_nc():
    if not _NC_CACHE:
        _NC_CACHE.append(build_bass())
    return _NC_CACHE[0]


LAST_RESULTS = []  # stash of the last BassKernelResults, for test harnesses


def _to_fp8(v, scale):
    return np.clip(v * scale, -240.0, 240.0).astype(ml_dtypes.float8_e4m3)


def make_in_maps(h, x, a_diag, p_vec, q_vec, b_mat):
    """Shard + lay out the full inputs into per-core in_maps."""
    h = np.asarray(h, dtype=np.float32)
    x = np.asarray(x, dtype=np.float32)
    a_diag = np.asarray(a_diag, dtype=np.float32)
    p_vec = np.asarray(p_vec, dtype=np.float32)
    q_vec = np.asarray(q_vec, dtype=np.float32)
    b_mat = np.asarray(b_mat, dtype=np.float32)

    bf = ml_dtypes.bfloat16
    NG = NT // NGROUP

    def pack_kT(arr):
        # [HIDDEN(k), M] -> [chunk, p, t, M]: k = (c*CH + t)*P + p
        M = arr.shape[1]
        return np.ascontiguousarray(
            arr.reshape(NCHUNK, CH, P, M).transpose(0, 2, 1, 3)
        )

    # b packed [g, c, p, t, n] so each (g, c) chunk is one contiguous block.
    b_f8 = _to_fp8(b_mat, SB)
    b_p = np.ascontiguousarray(
        b_f8.reshape(NCHUNK, CH, P, NG, NGROUP * P).transpose(3, 0, 2, 1, 4)
    )
    # hq path in fp8 DoubleRow: hq_psum = (SX*h)@(SB*q) = SOUT*(h@q), which
    # matches the SOUT scale of the main x@b product; pT stays bf16 unscaled.
    # The host divides SOUT back out after gather.
    q_pad = np.zeros((HIDDEN, RPAD), dtype=np.float32)
    q_pad[:, :RANK] = q_vec
    q8_f8 = np.ascontiguousarray(
        _to_fp8(q_pad, SB).reshape(KT, P, RPAD).transpose(1, 0, 2)
    )
    pT_bf = np.ascontiguousarray(p_vec.T.astype(bf))
    # a_r[p, t] = SOUT * (1 + a_diag[t*128 + p])
    a_r = np.ascontiguousarray((SOUT * (1.0 + a_diag)).reshape(NT, P).T)

    in_maps = []
    for c in range(N_CORES):
        sl = slice(c * MB, (c + 1) * MB)
        hT_c = h[sl].T
        in_maps.append(
            {
                "b": b_p,
                "xT": pack_kT(_to_fp8(x[sl].T, SX)),
                "hT": pack_kT(hT_c.astype(bf)),
                "hT8": pack_kT(_to_fp8(hT_c, SX)),
                "q8": q8_f8,
                "pT": pT_bf,
                "a_r": a_r,
            }
        )
    return in_maps


def _axon_device_reset():
    """Best-effort heal of a wedged axon-tunneled device (NRT_EXEC_UNIT_
    UNRECOVERABLE). No-op when the axon .so isn't present."""
    try:
        import ctypes

        lib = ctypes.CDLL("/opt/axon/libaxon_pjrt.so")
        lib.axon_reset.restype = ctypes.c_int64
        lib.axon_reset()
    except Exception:
        pass


def kernel(h, x, a_diag, p_vec, q_vec, b_mat, trace=False):
    nc = _get_nc()
    in_maps = make_in_maps(h, x, a_diag, p_vec, q_vec, b_mat)
    try:
        res = run_bass_kernel_spmd(
            nc, in_maps, core_ids=list(range(N_CORES)), trace=trace
        )
    except Exception as e:
        if "UNRECOVERABLE" not in str(e) and "UNAVAILABLE" not in str(e):
            raise
        _axon_device_reset()
        res = run_bass_kernel_spmd(
            nc, in_maps, core_ids=list(range(N_CORES)), trace=trace
        )
    LAST_RESULTS.clear()
    LAST_RESULTS.append(res)

    out = np.empty((BATCH, HIDDEN), dtype=np.float32)
    inv = np.float32(1.0 / SOUT)
    for c in range(N_CORES):
        out[c * MB : (c + 1) * MB, :] = res.results[c]["outT"].T * inv
    return out
